# revision 21
# baseline (speedup 1.0000x reference)
"""Causal cross-attention (q=k=v projection) kernel for Trainium2, 8 cores.

Problem (hardcoded): B=8, L=2048, D=1024, fp32.
    q = x @ Wq.T; k = enc @ Wq.T; v = k
    s = causal(q @ k.T / sqrt(D)); out = softmax(s) @ v

Sharding: data-parallel over batch, one batch element per NeuronCore, Wq
replicated.  All on-chip compute in bf16 (fp32 PSUM accumulation); the host
pre-transposes inputs so every matmul has its contraction dim on partitions:

per core, with xT/encT/wqT host-prepared ([D,L]/[D,L]/[D,D] bf16):
  Kkh[k,h] = sum_d encT[d,k] * wqT[d,h]         (lhsT=encT tile, rhs=wqT)
  KT[h,k]  = DMA-XBAR transpose of Kkh          (DMA engines, not PE)
  QT[h,q]  = sum_d wqT[d,h] * xT[d,q]           (lhsT=wqT tile, rhs=xT tile)
  ST[k,q]  = sum_h KT[h,k] * QT[h,q]            (scores, transposed)
  P[k,q]   = exp(ST/32) * causal_mask           (no max-subtraction: |s|<~6)
  rsT[q,1] = P.T @ ones   per 128-q chunk       (PE, accumulated over k chunks)
  O[q,h]   = sum_k P[k,q].T ... = lhsT=P chunk, rhs=Kkh  (q on partitions)
  out      = O * recip(rsT)  per-partition scale fused into the ACT
             PSUM->SBUF copy; stored bf16, host upcasts to f32.
Output is produced in natural [L, D] layout (no host transpose).
"""

import sys

for _p in ("/opt/trn_rl_repo",):
    if _p not in sys.path:
        sys.path.insert(0, _p)

import numpy as np
import ml_dtypes

import concourse.bass as bass
import concourse.tile as tile
from concourse import bacc, mybir
from concourse.masks import make_upper_triangular

B, L, D = 8, 2048, 1024
P = 128                    # partitions
ND = D // P                # 8 d-chunks
NH = D // P                # 8 h-chunks
NK = L // P                # 16 k-chunks
QW = 512                   # q block width
NJ = L // QW               # 4 q blocks
NQC = QW // P              # 4 q chunks per block
SCALE = 1.0 / np.sqrt(np.float32(D))   # 1/32

BF16 = mybir.dt.bfloat16
F32 = mybir.dt.float32

_CACHED = {}


def build_program():
    """Build the per-core Bass/Tile program (same for all 8 cores)."""
    # Bacc (not raw Bass): its compile() splits multi-sem waits into
    # EventSemaphore instructions — walrus encodes at most one wait per
    # instruction, so raw Bass programs with Tile-generated multi-waits
    # fail codegen ("Too many sync wait commands").
    nc = bacc.Bacc("TRN2", target_bir_lowering=False, debug=False, num_devices=B)

    xt = nc.declare_dram_parameter("xt", [D, L], BF16, isOutput=False).ap()
    enct = nc.declare_dram_parameter("enct", [D, L], BF16, isOutput=False).ap()
    wqt = nc.declare_dram_parameter("wqt", [D, D], BF16, isOutput=False).ap()
    # separate output tensor per (q-block, q-chunk, h-half) store: a single
    # shared output tensor makes Tile chain stores with cross-queue WAW waits,
    # and 2-wait DMAs don't fit the direct2d descriptor (walrus error).
    outs = [
        [[nc.declare_dram_parameter(f"o_{J}_{qc}_{hh}", [P, QW], BF16,
                                    isOutput=True).ap()
          for hh in range(2)]
         for qc in range(NQC)]
        for J in range(NJ)
    ]

    with tile.TileContext(nc) as tc:
        _emit(nc, tc, xt, enct, wqt, outs)
    nc.compile()
    _check_dma_waits(nc)
    return nc


def _check_dma_waits(nc):
    """HWDGE direct2d descriptors encode only ONE wait; multi-wait DMAs make
    walrus fail codegen. SWDGE (gpsimd queue) triggers run on the Pool
    sequencer where walrus can split waits, so only check HWDGE queues."""
    fn = nc.m.functions[0]
    bad = [
        (i.name, i.queue, [(w.ant_name, w.wait_value) for w in i.sync_info.on_wait])
        for blk in fn.blocks for i in blk.instructions
        if type(i).__name__ in ("InstDMACopy", "InstDmaTransposeAnt")
        and "DynamicHW" in (getattr(i, "queue", None) or "")
        and len(i.sync_info.on_wait) > 1
    ]
    assert not bad, f"multi-wait HWDGE DMAs: {bad[:4]} (+{len(bad)-4} more)"


def _emit(nc, tc, xt, enct, wqt, outs):
    from contextlib import ExitStack

    ctx = ExitStack()
    consts = ctx.enter_context(tc.tile_pool(name="consts", bufs=1))
    kt_pool = ctx.enter_context(tc.tile_pool(name="ktp", bufs=1))
    kkh_pool = ctx.enter_context(tc.tile_pool(name="kkhp", bufs=1))
    enc_pool = ctx.enter_context(tc.tile_pool(name="encp", bufs=4))
    x_pool = ctx.enter_context(tc.tile_pool(name="xp", bufs=4))
    qt_pool = ctx.enter_context(tc.tile_pool(name="qtp", bufs=2))
    p_pool = ctx.enter_context(tc.tile_pool(name="pp", bufs=18))
    o_pool = ctx.enter_context(tc.tile_pool(name="op", bufs=4))
    misc = ctx.enter_context(tc.tile_pool(name="misc", bufs=2))
    mm_psum = ctx.enter_context(tc.tile_pool(name="mmps", bufs=6, space="PSUM"))
    rs_psum = ctx.enter_context(tc.tile_pool(name="rsps", bufs=2, space="PSUM"))

    # ---- constants + all input DMAs up front, in order of first use ----
    # (cost model serializes DMA transfers: order = priority)
    wq_sb = consts.tile([P, ND, D], BF16)       # wqT[d,h] as [p, d_chunk, h]
    enc_tiles = []
    for kp in range(NJ):                        # 4 k panels of 512
        enc_tiles.append(enc_pool.tile([P, ND, QW], BF16, tag="enc", name=f"enc_t{kp}"))
    # Startup-critical loads interleaved at chunk granularity: the first
    # K-proj groups (panel 0, hh=0) need only the low-h half of Wq plus the
    # matching enc chunk, so compute starts after ~1 MiB instead of 2.5 MiB.
    for c in range(ND):
        nc.sync.dma_start(out=wq_sb[:, c, 0:QW],
                          in_=wqt[c * P:(c + 1) * P, 0:QW])
        nc.sync.dma_start(
            out=enc_tiles[0][:, c, :],
            in_=enct[c * P:(c + 1) * P, 0:QW],
        )
    for c in range(ND):                         # high-h half of Wq
        nc.sync.dma_start(out=wq_sb[:, c, QW:D],
                          in_=wqt[c * P:(c + 1) * P, QW:D])
    for kp in range(1, NJ):                     # panels 1-3: one DMA each
        nc.sync.dma_start(
            out=enc_tiles[kp][:, :, :],
            in_=enct[:, kp * QW:(kp + 1) * QW].rearrange(
                "(c p) w -> p c w", p=P),
        )
    x_tiles = []
    for J in range(NJ):
        x_tiles.append(x_pool.tile([P, ND, QW], BF16, tag="x", name=f"x_t{J}"))
        nc.sync.dma_start(
            out=x_tiles[J][:, :, :],
            in_=xt[:, J * QW:(J + 1) * QW].rearrange("(c p) w -> p c w", p=P),
        )

    ut_mask = consts.tile([P, P], BF16)         # 1 where q_loc >= k_loc
    make_upper_triangular(nc, ut_mask, val=1.0, diag=True)
    ones_col = consts.tile([P, 1], BF16)
    nc.vector.memset(ones_col, 1.0)

    # ---- phase A: K = enc @ Wq.T; KT via DMA-XBAR transpose ----
    kt_sb = kt_pool.tile([P, NH, L], BF16)       # KT[h,k]: [p, h_chunk, k]
    kkh_sb = kkh_pool.tile([P, NK, D], BF16)     # Kkh[k,h]: [p, k_chunk, h]

    def _kproj_group(enc_tile, kt_g, ktl, hh):
        ps_k = mm_psum.tile([P, QW], F32, tag="mm", name="ps_k")
        for c in range(ND):
            nc.tensor.matmul(
                ps_k,
                lhsT=enc_tile[:, c, ktl * P:(ktl + 1) * P],
                rhs=wq_sb[:, c, hh * QW:(hh + 1) * QW],
                start=(c == 0),
                stop=(c == ND - 1),
            )
        nc.scalar.copy(out=kkh_sb[:, kt_g, hh * QW:(hh + 1) * QW], in_=ps_k)

    def _kt_transpose(kt_g):
        # transpose the whole [128k, 1024h] row to KT in ONE DMA-XBAR
        # instruction: out[hw, ch, k] = in[k, ch*128+hw].  (3D out AP;
        # the XBAR interp computes in.reshape(reversed(out.shape)).T.)
        nc.sync.dma_start_transpose(
            out=kt_sb[:, :, kt_g * P:(kt_g + 1) * P],
            in_=kkh_sb[:, kt_g, :],
        )

    # panel 0: hh-outer so the first groups need only the low-h Wq half
    for hh in range(D // QW):
        for ktl in range(QW // P):
            _kproj_group(enc_tiles[0], ktl, ktl, hh)
            if hh == 1:
                _kt_transpose(ktl)
    for kp in range(1, NJ):
        for ktl in range(QW // P):               # 4 k tiles of 128 in panel
            kt_g = kp * (QW // P) + ktl          # global k chunk index
            for hh in range(D // QW):            # 2 h halves of 512
                _kproj_group(enc_tiles[kp], kt_g, ktl, hh)
            _kt_transpose(kt_g)

    # ---- phase B: per q block of 512 ----
    for J in range(NJ):
        # QT[h, q] for this block (PSUM->SBUF copies on DVE)
        qt_sb = qt_pool.tile([P, NH, QW], BF16, tag="qt")
        for ch in range(NH):
            ps_q = mm_psum.tile([P, QW], F32, tag="mm")
            for c in range(ND):
                nc.tensor.matmul(
                    ps_q,
                    lhsT=wq_sb[:, c, ch * P:(ch + 1) * P],
                    rhs=x_tiles[J][:, c, :],
                    start=(c == 0),
                    stop=(c == ND - 1),
                )
            nc.vector.tensor_copy(out=qt_sb[:, ch, :], in_=ps_q)

        ncnk = 4 * J + 4                         # num k chunks with any valid q
        p_tiles = []
        col0s = []
        for c in range(ncnk):
            j = c - 4 * J                        # >=0 on diagonal chunks
            col0 = max(0, P * j)
            col0s.append(col0)
            ps_s = mm_psum.tile([P, QW], F32, tag="mm")
            for ch in range(NH):
                nc.tensor.matmul(
                    ps_s[:, col0:QW],
                    lhsT=kt_sb[:, ch, c * P:(c + 1) * P],
                    rhs=qt_sb[:, ch, col0:QW],
                    start=(ch == 0),
                    stop=(ch == NH - 1),
                )
            p_t = p_pool.tile([P, QW], BF16, tag="p")
            nc.scalar.activation(
                out=p_t[:, col0:QW],
                in_=ps_s[:, col0:QW],
                func=mybir.ActivationFunctionType.Exp,
                scale=float(SCALE),
            )
            if j >= 0:                           # causal mask on diagonal block
                nc.vector.tensor_mul(
                    out=p_t[:, col0:col0 + P],
                    in0=p_t[:, col0:col0 + P],
                    in1=ut_mask,
                )
            p_tiles.append(p_t)

        # O[q,h] = sum_k P[k,q]^T Kkh[k,h]; normalize fused into ACT copy.
        # rowsum rsT[q,1] = P.T @ ones rides along inside the hh=0 group:
        # each rs matmul reuses the exact lhsT of the preceding out matmul,
        # so the PE skips the redundant LDWEIGHTS (near-free on HW).
        # qc descending: the biggest accumulation starts first, so the kernel
        # tail is gated by the smallest group instead of the largest.
        recip_sb = misc.tile([P, NQC], F32, tag="recip")
        for qc in reversed(range(NQC)):
            nck = 4 * J + qc + 1
            for hh in range(2):
                # split the kernel's very last group into two 256-wide psum
                # groups: the first half's norm+store overlaps the second
                # half's matmuls, shortening the end-of-kernel tail.
                nsplit = 2 if (J == NJ - 1 and qc == 0 and hh == 1) else 1
                w = QW // nsplit
                for sp in range(nsplit):
                    ps_o = mm_psum.tile([P, w], F32, tag="mm", name="ps_o")
                    rs_ps = (rs_psum.tile([P, 1], F32, tag="rs", name="rs_ps")
                             if hh == 0 and sp == 0 else None)
                    b0 = hh * QW + sp * w
                    for c in range(nck):
                        nc.tensor.matmul(
                            ps_o,
                            lhsT=p_tiles[c][:, qc * P:(qc + 1) * P],
                            rhs=kkh_sb[:, c, b0:b0 + w],
                            start=(c == 0),
                            stop=(c == nck - 1),
                        )
                        if rs_ps is not None:
                            nc.tensor.matmul(
                                rs_ps,
                                lhsT=p_tiles[c][:, qc * P:(qc + 1) * P],
                                rhs=ones_col,
                                start=(c == 0),
                                stop=(c == nck - 1),
                            )
                    if rs_ps is not None:
                        nc.vector.reciprocal(
                            out=recip_sb[:, qc:qc + 1], in_=rs_ps)
                    o_sb = o_pool.tile([P, w], BF16, tag="o", name="o_sb")
                    nc.scalar.activation(
                        out=o_sb,
                        in_=ps_o,
                        func=mybir.ActivationFunctionType.Copy,
                        scale=recip_sb[:, qc:qc + 1],
                    )
                    # store from the ACT queue: it directly follows the
                    # producing normalize in the same FIFO, so it never blocks
                    # on the SP queue's transpose backlog (which would pin
                    # o_pool slots and stall ACT -> exp -> PE).
                    nc.scalar.dma_start(
                        out=outs[J][qc][hh][:, sp * w:(sp + 1) * w], in_=o_sb)
    ctx.close()


def _get_program():
    if "nc" not in _CACHED:
        _CACHED["nc"] = build_program()
    return _CACHED["nc"]


def kernel(enc_outputs: np.ndarray, x: np.ndarray, Wq: np.ndarray) -> np.ndarray:
    from concourse.bass_utils import run_bass_kernel_spmd

    nc = _get_program()
    bf16 = ml_dtypes.bfloat16
    wqt = np.ascontiguousarray(np.asarray(Wq, dtype=np.float32).T).astype(bf16)
    in_maps = []
    for b in range(B):
        in_maps.append({
            "xt": np.ascontiguousarray(np.asarray(x[b], np.float32).T).astype(bf16),
            "enct": np.ascontiguousarray(
                np.asarray(enc_outputs[b], np.float32).T).astype(bf16),
            "wqt": wqt,
        })
    res = run_bass_kernel_spmd(nc, in_maps, list(range(B)))
    _CACHED["last_result"] = res
    out = np.empty((B, L, D), dtype=np.float32)
    for b in range(B):
        for J in range(NJ):
            for qc in range(NQC):
                r0 = J * QW + qc * P
                for hh in range(2):
                    out[b, r0:r0 + P, hh * QW:(hh + 1) * QW] = \
                        res.results[b][f"o_{J}_{qc}_{hh}"].astype(np.float32)
    return out
